# revision 8
# baseline (speedup 1.0000x reference)
"""Biased axial (tied) attention kernel for 8 Trainium2 NeuronCores.

Sharding: the score einsum contracts over the first L axis (n) of the
LN'd/transposed pair tensor.  Each core takes 48 of the 384 n-rows,
computes partial scores A[h,i,j] for ALL (i,j), and the partials are
summed with an on-chip AllReduce (2.4 MB).  The per-(i,j) bias
projection is sharded over i and exchanged with a small AllGather.
After the reduce every core redundantly softmaxes the full score
tensor and computes output columns k in its own n-shard (out[:,k] only
needs attn rows (all i) and locally-projected V rows), so the output
rows of the final (transposed) result are shard-contiguous.

Compute layout: LayerNorm runs position-major (positions on SBUF
partitions) with bn_stats; normalized bf16 tiles are flipped to
feature-major via DMA-xbar transposes; all matmuls run in bf16 with
fp32 PSUM accumulation.  Scores use K=32 row-tiled accumulation (4
heads concurrent in the PE array); the output einsum uses M=32
col-tiled matmuls per head from a j-partition-major V.
"""

import functools
import math
from contextlib import ExitStack

import numpy as np
import ml_dtypes

import concourse.bacc as bacc
import concourse.mybir as mybir
from concourse.bass_utils import run_bass_kernel_spmd
from concourse.tile import TileContext

N_CORES = 8
L = 384
D = 128
H = 4
DH = 32
NL = L // N_CORES          # 48 rows per core
NCHUNK = L // 128          # 3
NPOS = L * NL              # 18432 positions per LN'd tensor slice
EPS = 1e-5

F32 = mybir.dt.float32
BF16 = mybir.dt.bfloat16
AX = mybir.AxisListType
ALU = mybir.AluOpType
ACTF = mybir.ActivationFunctionType

RG = [list(range(N_CORES))]


def _emit_ln(nc, pools, src, layout, lnfm, s6tag):
    """LayerNorm `src` (DRAM f32) position-major, write bf16 feature-major
    into SBUF tile `lnfm` [128, NPOS] (pos = inner*384 + outer).

    layout 'oi': src [L, NL, D] (partition axis 0, group axis 1)
    layout 'io': src [NL, L, D] (partition axis 1, group axis 0)
    """
    s6p, mvp, xrawp = pools
    s6 = s6p.tile([128, NCHUNK, 12, 4, 6], F32, tag=s6tag)
    mean = mvp.tile([128, NCHUNK * 48], F32, tag="mean")
    negm = mvp.tile([128, NCHUNK * 48], F32, tag="negm")
    tA = mvp.tile([128, NCHUNK * 48], F32, tag="tA")
    tB = mvp.tile([128, NCHUNK * 48], F32, tag="tB")
    rs = mvp.tile([128, NCHUNK * 48], F32, tag="rs")
    for cc in range(NCHUNK):
        xts = []
        for t4 in range(12):
            xt = xrawp.tile([128, 4, D], BF16, tag=f"xraw{t4}")
            if layout == "oi":
                ap = src[cc * 128:(cc + 1) * 128, t4 * 4:(t4 + 1) * 4, :]
            else:
                ap = src[t4 * 4:(t4 + 1) * 4, cc * 128:(cc + 1) * 128, :] \
                    .rearrange("k i d -> i k d")
            nc.gpsimd.dma_start(out=xt[:], in_=ap)  # f32 -> bf16 cast
            for g in range(4):
                nc.vector.bn_stats(out=s6[:, cc, t4, g, :], in_=xt[:, g, :])
            xts.append(xt)
        # batched stats post-processing for this cc: columns n = t4*4+g
        # bn_stats 6-tuple = (cnt, mean, cnt*var) of even / odd elements.
        sl = slice(cc * 48, (cc + 1) * 48)
        me = s6[:, cc, :, :, 1].rearrange("p a b -> p (a b)")
        mo = s6[:, cc, :, :, 4].rearrange("p a b -> p (a b)")
        cve = s6[:, cc, :, :, 2].rearrange("p a b -> p (a b)")
        cvo = s6[:, cc, :, :, 5].rearrange("p a b -> p (a b)")
        nc.vector.tensor_add(out=tA[:, sl], in0=me, in1=mo)
        nc.vector.tensor_scalar_mul(mean[:, sl], tA[:, sl], 0.5)
        nc.vector.tensor_scalar_mul(negm[:, sl], mean[:, sl], -1.0)
        # var = (cve+cvo)/128 + (me^2+mo^2)/2 - mean^2
        nc.vector.tensor_add(out=tA[:, sl], in0=cve, in1=cvo)
        nc.vector.tensor_scalar_mul(tA[:, sl], tA[:, sl], 1.0 / 128.0)
        nc.vector.tensor_mul(out=tB[:, sl], in0=me, in1=me)
        nc.vector.scalar_tensor_tensor(
            out=tA[:, sl], in0=tB[:, sl], scalar=0.5, in1=tA[:, sl],
            op0=ALU.mult, op1=ALU.add)
        nc.vector.tensor_mul(out=tB[:, sl], in0=mo, in1=mo)
        nc.vector.scalar_tensor_tensor(
            out=tA[:, sl], in0=tB[:, sl], scalar=0.5, in1=tA[:, sl],
            op0=ALU.mult, op1=ALU.add)
        nc.vector.tensor_mul(out=tB[:, sl], in0=mean[:, sl], in1=mean[:, sl])
        nc.vector.tensor_sub(out=tA[:, sl], in0=tA[:, sl], in1=tB[:, sl])
        # rs = 1/sqrt(var+eps)
        nc.vector.tensor_scalar_add(tA[:, sl], tA[:, sl], EPS)
        nc.scalar.sqrt(out=tB[:, sl], in_=tA[:, sl])
        nc.vector.reciprocal(out=rs[:, sl], in_=tB[:, sl])
        # apply (in place) + xbar transpose to feature-major
        for t4 in range(12):
            for g in range(4):
                n = t4 * 4 + g
                col = cc * 48 + n
                nc.vector.tensor_scalar(
                    out=xts[t4][:, g, :], in0=xts[t4][:, g, :],
                    scalar1=negm[:, col:col + 1], scalar2=rs[:, col:col + 1],
                    op0=ALU.add, op1=ALU.mult)
                nc.sync.dma_start(
                    out=lnfm[:, n * L + cc * 128: n * L + (cc + 1) * 128],
                    in_=xts[t4][:, g, :], transpose=True)


@functools.lru_cache(maxsize=4)
def build_program(has_bo: bool):
    nc = bacc.Bacc(num_devices=N_CORES)

    xr = nc.declare_dram_parameter("xr", [L, NL, D], F32, isOutput=False)
    xc = nc.declare_dram_parameter("xc", [NL, L, D], F32, isOutput=False)
    xb = nc.declare_dram_parameter("xb", [L, NL, D], F32, isOutput=False)
    wq = nc.declare_dram_parameter("wq", [D, D], BF16, isOutput=False)
    wk = nc.declare_dram_parameter("wk", [D, D], BF16, isOutput=False)
    wv = nc.declare_dram_parameter("wv", [D, D], BF16, isOutput=False)
    wg = nc.declare_dram_parameter("wg", [D, D], BF16, isOutput=False)
    wo = nc.declare_dram_parameter("wo", [D, D], BF16, isOutput=False)
    wb = nc.declare_dram_parameter("wb", [D, H], BF16, isOutput=False)
    cq = nc.declare_dram_parameter("cq", [D, 1], F32, isOutput=False)
    ck = nc.declare_dram_parameter("ck", [D, 1], F32, isOutput=False)
    cv = nc.declare_dram_parameter("cv", [D, 1], F32, isOutput=False)
    cg = nc.declare_dram_parameter("cg", [D, 1], F32, isOutput=False)
    cb = nc.declare_dram_parameter("cb", [4, 1], F32, isOutput=False)
    bo_b = nc.declare_dram_parameter("bo_b", [D, D], F32, isOutput=False)
    out = nc.declare_dram_parameter("out", [NL, L, D], F32, isOutput=True)

    a_part = nc.dram_tensor("a_part", [H, L, L], F32)
    a_red = nc.dram_tensor("a_red", [H, L, L], F32, addr_space="Shared")
    bp_part = nc.dram_tensor("bp_part", [H, NL, L], F32)
    bp_gath = nc.dram_tensor("bp_gath", [N_CORES, H, NL, L], F32,
                             addr_space="Shared")

    with TileContext(nc) as tc, ExitStack() as es:
        cpool = es.enter_context(tc.tile_pool(name="consts", bufs=1))
        wq_sb = cpool.tile([D, D], BF16, tag="wq")
        wk_sb = cpool.tile([D, D], BF16, tag="wk")
        wv_sb = cpool.tile([D, D], BF16, tag="wv")
        wg_sb = cpool.tile([D, D], BF16, tag="wg")
        wo_sb = cpool.tile([D, D], BF16, tag="wo")
        wb_sb = cpool.tile([D, H], BF16, tag="wb")
        cq_sb = cpool.tile([D, 1], F32, tag="cq")
        ck_sb = cpool.tile([D, 1], F32, tag="ck")
        cv_sb = cpool.tile([D, 1], F32, tag="cv")
        cg_sb = cpool.tile([D, 1], F32, tag="cg")
        for t, s in [(wq_sb, wq), (wk_sb, wk), (wv_sb, wv), (wg_sb, wg),
                     (wo_sb, wo), (wb_sb, wb), (cq_sb, cq), (ck_sb, ck),
                     (cv_sb, cv), (cg_sb, cg)]:
            nc.sync.dma_start(out=t[:], in_=s[:])
        if has_bo:
            bo_sb = cpool.tile([D, D], F32, tag="bo")
            nc.sync.dma_start(out=bo_sb[:], in_=bo_b[:])

        s6p = es.enter_context(tc.tile_pool(name="s6", bufs=1))
        mvp = es.enter_context(tc.tile_pool(name="mv", bufs=1))
        xrawp = es.enter_context(tc.tile_pool(name="xraw", bufs=1))
        lnp = es.enter_context(tc.tile_pool(name="lnfm", bufs=1))
        bigp = es.enter_context(tc.tile_pool(name="big", bufs=1))
        ln_pools = (s6p, mvp, xrawp)

        # ---- phase 1: LN of pair rows (n-shard) -> lnfm1
        lnfm1 = lnp.tile([128, NPOS], BF16, tag="lnfm")
        _emit_ln(nc, ln_pools, xr, "oi", lnfm1, "s6")

        # ---- phase 2: q/k/v projections (feature-major)
        q_sb = bigp.tile([128, NPOS], BF16, tag="bigq")
        k_sb = bigp.tile([128, NPOS], BF16, tag="bigk")
        v_pm = bigp.tile([128, NCHUNK, NL, D], BF16, tag="bigvpm")
        with tc.tile_pool(name="ppsum", bufs=4, space="PSUM") as ppsum, \
             tc.tile_pool(name="vrot", bufs=4) as vrotp:
            for dst, w_sb, c_sb, eng in ((q_sb, wq_sb, cq_sb, "act"),
                                         (k_sb, wk_sb, ck_sb, "dve")):
                for ch in range(NPOS // 512):
                    ps = ppsum.tile([128, 512], F32, tag="pps")
                    sl = slice(ch * 512, (ch + 1) * 512)
                    nc.tensor.matmul(ps[:], lhsT=w_sb[:], rhs=lnfm1[:, sl],
                                     start=True, stop=True)
                    if eng == "act":
                        nc.scalar.activation(dst[:, sl], ps[:], ACTF.Identity,
                                             bias=c_sb[:, 0:1])
                    else:
                        nc.vector.tensor_scalar_add(dst[:, sl], ps[:],
                                                    c_sb[:, 0:1])
            # v: per-n chunks so xbar transposes align to pure-j windows
            for n in range(NL):
                ps = ppsum.tile([128, L], F32, tag="vps")
                sl = slice(n * L, (n + 1) * L)
                nc.tensor.matmul(ps[:], lhsT=wv_sb[:], rhs=lnfm1[:, sl],
                                 start=True, stop=True)
                vr = vrotp.tile([128, L], BF16, tag="vrot")
                nc.vector.tensor_scalar_add(vr[:], ps[:], cv_sb[:, 0:1])
                for jc in range(NCHUNK):
                    nc.sync.dma_start(out=v_pm[:, jc, n, :],
                                      in_=vr[:, jc * 128:(jc + 1) * 128],
                                      transpose=True)

        # ---- phase 3: LN bias shard -> lnfm2; bproj position-major
        lnfm2 = lnp.tile([128, NPOS], BF16, tag="lnfm")
        _emit_ln(nc, ln_pools, xb, "oi", lnfm2, "s6")
        bp_sb = bigp.tile([128, NPOS // 128, H], F32, tag="bpsb")
        with tc.tile_pool(name="bpps", bufs=4, space="PSUM") as bpps:
            for tg in range(NPOS // 512):
                ps = bpps.tile([128, 4, H], F32, tag="bpps")
                for c4 in range(4):
                    t = tg * 4 + c4
                    nc.tensor.matmul(ps[:, c4, :],
                                     lhsT=lnfm2[:, t * 128:(t + 1) * 128],
                                     rhs=wb_sb[:], start=True, stop=True)
                nc.vector.tensor_copy(bp_sb[:, tg * 4:(tg + 1) * 4, :], ps[:])
        # stage to DRAM: bp_part[h, il, j]; chunk t=(il, jc), partition=j%128
        for h in range(H):
            nc.sync.dma_start(
                out=bp_part[h].rearrange("il (jc p) -> p il jc", p=128),
                in_=bp_sb[:, :, h].rearrange("p (il jc) -> p il jc", jc=NCHUNK))

        # ---- phase 4: scores A[i, j] per head, K=32 row-tiled over n
        with tc.tile_pool(name="apsum", bufs=2, space="PSUM") as apsum, \
             tc.tile_pool(name="asb", bufs=2) as asbp:
            for ic in range(NCHUNK):
                aps = [apsum.tile([128, L], F32, tag=f"A{h}", name=f"A{h}") for h in range(H)]
                for n in range(NL):
                    for h in range(H):
                        nc.tensor.matmul(
                            aps[h][:],
                            lhsT=q_sb[32 * h:32 * (h + 1),
                                      n * L + ic * 128: n * L + (ic + 1) * 128],
                            rhs=k_sb[32 * h:32 * (h + 1), n * L:(n + 1) * L],
                            start=(n == 0), stop=(n == NL - 1),
                            tile_position=(32 * h, 0))
                a_sb = asbp.tile([128, H, L], F32, tag="asb")
                for h in range(H):
                    if h % 2 == 0:
                        nc.vector.tensor_copy(a_sb[:, h, :], aps[h][:])
                    else:
                        nc.scalar.copy(a_sb[:, h, :], aps[h][:])
                nc.sync.dma_start(
                    out=a_part[:, ic * 128:(ic + 1) * 128, :]
                        .rearrange("h i j -> i h j"),
                    in_=a_sb[:])

        # ---- phase 5: collectives
        nc.gpsimd.collective_compute(
            "AllReduce", ALU.add, replica_groups=RG,
            ins=[a_part[:]], outs=[a_red[:]])
        nc.gpsimd.collective_compute(
            "AllGather", ALU.bypass, replica_groups=RG,
            ins=[bp_part[:]], outs=[bp_gath[:]])

        # ---- phase 6: gate (overlaps collectives via deps)
        lnfm3 = lnp.tile([128, NPOS], BF16, tag="lnfm")
        _emit_ln(nc, ln_pools, xc, "io", lnfm3, "s6")
        gate_sb = bigp.tile([128, NPOS], BF16, tag="bigq")
        with tc.tile_pool(name="gpsum", bufs=4, space="PSUM") as gpsum:
            for ch in range(NPOS // 512):
                ps = gpsum.tile([128, 512], F32, tag="gps")
                sl = slice(ch * 512, (ch + 1) * 512)
                nc.tensor.matmul(ps[:], lhsT=wg_sb[:], rhs=lnfm3[:, sl],
                                 start=True, stop=True)
                nc.scalar.activation(gate_sb[:, sl], ps[:], ACTF.Sigmoid,
                                     bias=cg_sb[:, 0:1])

        # ---- phase 7: softmax (redundant on every core) + attn transpose
        attnT = bigp.tile([128, NCHUNK, H, L], BF16, tag="bigattnT")
        with tc.tile_pool(name="smp", bufs=1) as smp, \
             tc.tile_pool(name="sms", bufs=2) as sms:
            for ic in range(NCHUNK):
                i0 = ic * 128
                a_ch = smp.tile([128, H, L], F32, tag="ach")
                nc.sync.dma_start(
                    out=a_ch[:],
                    in_=a_red[:, i0:i0 + 128, :].rearrange("h i j -> i h j"))
                bp_ch = smp.tile([128, H, L], F32, tag="bpch")
                for cpr in range(N_CORES):
                    lo = max(cpr * NL, i0)
                    hi = min((cpr + 1) * NL, i0 + 128)
                    if lo >= hi:
                        continue
                    nc.sync.dma_start(
                        out=bp_ch[lo - i0:hi - i0, :, :],
                        in_=bp_gath[cpr, :, lo - cpr * NL:hi - cpr * NL, :]
                            .rearrange("h il j -> il h j"))
                # A += bp (+cb per head, baked as floats = 0 in practice)
                nc.vector.tensor_add(out=a_ch[:], in0=a_ch[:], in1=bp_ch[:])
                nm = sms.tile([128, H], F32, tag="nm")
                nc.vector.tensor_reduce(out=nm[:], in_=a_ch[:], axis=AX.X,
                                        op=ALU.max)
                nmn = sms.tile([128, H], F32, tag="nmn")
                nc.vector.tensor_scalar_mul(nmn[:], nm[:], -1.0)
                e_ch = smp.tile([128, H, L], F32, tag="bpch")
                ssum = sms.tile([128, H], F32, tag="ssum")
                for h in range(H):
                    nc.scalar.activation(e_ch[:, h, :], a_ch[:, h, :], ACTF.Exp,
                                         bias=nmn[:, h:h + 1],
                                         accum_out=ssum[:, h:h + 1])
                rsum = sms.tile([128, H], F32, tag="rsum")
                nc.vector.reciprocal(out=rsum[:], in_=ssum[:])
                at_ch = smp.tile([128, H, L], BF16, tag="ach")
                for h in range(H):
                    nc.vector.tensor_scalar_mul(at_ch[:, h, :], e_ch[:, h, :],
                                                rsum[:, h:h + 1])
                for h in range(H):
                    for jc in range(NCHUNK):
                        nc.sync.dma_start(
                            out=attnT[:, jc, h, i0:i0 + 128],
                            in_=at_ch[:, h, jc * 128:(jc + 1) * 128],
                            transpose=True)

        # ---- phase 8: output einsum (col-tiled by head) + gate + out proj
        with tc.tile_pool(name="opsum", bufs=2, space="PSUM") as opsum, \
             tc.tile_pool(name="fpsum", bufs=2, space="PSUM") as fpsum, \
             tc.tile_pool(name="ogp", bufs=3) as ogp, \
             tc.tile_pool(name="fsbp", bufs=3) as fsbp:
            for k in range(NL):
                ops_t = opsum.tile([128, L], F32, tag="ops")
                for jc in range(NCHUNK):
                    for h in range(H):
                        nc.tensor.matmul(
                            ops_t[32 * h:32 * (h + 1), :],
                            lhsT=v_pm[:, jc, k, 32 * h:32 * (h + 1)],
                            rhs=attnT[:, jc, h, :],
                            start=(jc == 0), stop=(jc == NCHUNK - 1),
                            tile_position=(0, 32 * h))
                og = ogp.tile([128, L], BF16, tag="og")
                nc.vector.scalar_tensor_tensor(
                    out=og[:], in0=ops_t[:], scalar=1.0,
                    in1=gate_sb[:, k * L:(k + 1) * L],
                    op0=ALU.mult, op1=ALU.mult)
                fps = fpsum.tile([128, NCHUNK, D], F32, tag="fps")
                for pc in range(NCHUNK):
                    nc.tensor.matmul(fps[:, pc, :],
                                     lhsT=og[:, pc * 128:(pc + 1) * 128],
                                     rhs=wo_sb[:], start=True, stop=True)
                fsb = fsbp.tile([128, NCHUNK, D], F32, tag="fsb")
                if has_bo:
                    for pc in range(NCHUNK):
                        nc.vector.tensor_add(out=fsb[:, pc, :],
                                             in0=fps[:, pc, :], in1=bo_sb[:])
                elif k % 2 == 0:
                    nc.vector.tensor_copy(fsb[:], fps[:])
                else:
                    nc.scalar.copy(fsb[:], fps[:])
                nc.sync.dma_start(
                    out=out[k].rearrange("(pc p) d -> p pc d", p=128),
                    in_=fsb[:])

    nc.compile()
    return nc


def _prep_inputs(pair, bias, ln_pair_w, ln_pair_b, ln_bias_w, ln_bias_b,
                 Wq, Wk, Wv, Wb, Wg, bg, Wo, bo):
    bf = ml_dtypes.bfloat16
    scaling = 1.0 / math.sqrt(DH)
    kscale = 1.0 / math.sqrt(L)
    wq_e = (ln_pair_w[:, None] * Wq * scaling).astype(bf)
    wk_e = (ln_pair_w[:, None] * Wk * kscale).astype(bf)
    wv_e = (ln_pair_w[:, None] * Wv).astype(bf)
    wg_e = (ln_pair_w[:, None] * Wg).astype(bf)
    wb_e = (ln_bias_w[:, None] * Wb).astype(bf)
    wo_e = Wo.astype(bf)
    cq_e = (ln_pair_b @ (Wq * scaling)).astype(np.float32).reshape(D, 1)
    ck_e = (ln_pair_b @ (Wk * kscale)).astype(np.float32).reshape(D, 1)
    cv_e = (ln_pair_b @ Wv).astype(np.float32).reshape(D, 1)
    cg_e = (bg + ln_pair_b @ Wg).astype(np.float32).reshape(D, 1)
    cb_e = (ln_bias_b @ Wb).astype(np.float32).reshape(H, 1)
    bo_f = np.asarray(bo, np.float32)
    has_bo = bool(np.any(bo_f != 0.0))
    bo_bcast = np.broadcast_to(bo_f, (D, D)).copy() if has_bo \
        else np.zeros((D, D), np.float32)

    common = dict(wq=wq_e, wk=wk_e, wv=wv_e, wg=wg_e, wo=wo_e, wb=wb_e,
                  cq=cq_e, ck=ck_e, cv=cv_e, cg=cg_e, cb=cb_e, bo_b=bo_bcast)
    in_maps = []
    for c in range(N_CORES):
        r0 = c * NL
        m = dict(common)
        m["xr"] = np.ascontiguousarray(pair[0, :, r0:r0 + NL, :], np.float32)
        m["xc"] = np.ascontiguousarray(pair[0, r0:r0 + NL, :, :], np.float32)
        m["xb"] = np.ascontiguousarray(bias[0, :, r0:r0 + NL, :], np.float32)
        in_maps.append(m)
    return in_maps, has_bo


TRACE = False
LAST_EXEC_NS = None
LAST_TRACE_DIR = None


def kernel(**inputs):
    global LAST_EXEC_NS, LAST_TRACE_DIR
    inputs = {k: np.asarray(v) for k, v in inputs.items()}
    in_maps, has_bo = _prep_inputs(**inputs)
    nc = build_program(has_bo)
    res = run_bass_kernel_spmd(nc, in_maps, list(range(N_CORES)), trace=TRACE)
    if TRACE:
        LAST_EXEC_NS = res.exec_time_ns
    full = np.concatenate([res.results[c]["out"] for c in range(N_CORES)],
                          axis=0)[None]
    return full.astype(np.float32)


if __name__ == "__main__":
    nc = build_program(False)
    print("build ok")


# revision 11
# speedup vs baseline: 2.0188x; 2.0188x over previous
"""Biased axial (tied) attention kernel for 8 Trainium2 NeuronCores.

Sharding: the score einsum contracts over the first L axis (n) of the
LN'd/transposed pair tensor.  Each core takes 48 of the 384 n-rows,
computes partial scores A[h,i,j] for ALL (i,j), and the partials are
summed with an on-chip AllReduce (2.4 MB).  The per-(i,j) bias
projection is sharded over i and exchanged with a small AllGather.
After the reduce every core redundantly softmaxes the full score
tensor and computes output columns k in its own n-shard (out[:,k] only
needs attn rows (all i) and locally-projected V rows), so the output
rows of the final (transposed) result are shard-contiguous.

Compute layout: LayerNorm runs position-major (positions on SBUF
partitions) with bn_stats; normalized bf16 tiles are flipped to
feature-major via DMA-xbar transposes; all matmuls run in bf16 with
fp32 PSUM accumulation.  Scores use K=32 row-tiled accumulation (4
heads concurrent in the PE array); the output einsum uses M=32
col-tiled matmuls per head from a j-partition-major V.
"""

import functools
import math
from contextlib import ExitStack

import numpy as np
import ml_dtypes

import concourse.bacc as bacc
import concourse.mybir as mybir
from concourse.bass_utils import run_bass_kernel_spmd
from concourse.tile import TileContext

N_CORES = 8
L = 384
D = 128
H = 4
DH = 32
NL = L // N_CORES          # 48 rows per core
NCHUNK = L // 128          # 3
NPOS = L * NL              # 18432 positions per LN'd tensor slice
EPS = 1e-5

F32 = mybir.dt.float32
BF16 = mybir.dt.bfloat16
AX = mybir.AxisListType
ALU = mybir.AluOpType
ACTF = mybir.ActivationFunctionType

RG = [list(range(N_CORES))]


def _emit_ln(nc, pools, src, layout, lnfm, s6tag):
    """LayerNorm `src` (DRAM f32) position-major, write bf16 feature-major
    into SBUF tile `lnfm` [128, NPOS] (pos = inner*384 + outer).

    layout 'oi': src [L, NL, D] (partition axis 0, group axis 1)
    layout 'io': src [NL, L, D] (partition axis 1, group axis 0)
    """
    s6p, mvp, xccp = pools
    s6 = s6p.tile([128, NCHUNK, 48, 6], F32, tag=s6tag)
    mean = mvp.tile([128, NCHUNK * 48], F32, tag="mean")
    negm = mvp.tile([128, NCHUNK * 48], F32, tag="negm")
    tA = mvp.tile([128, NCHUNK * 48], F32, tag="tA")
    tB = mvp.tile([128, NCHUNK * 48], F32, tag="tB")
    rs = mvp.tile([128, NCHUNK * 48], F32, tag="rs")
    for cc in range(NCHUNK):
        xcc = xccp.tile([128, 48, D], BF16, tag="xcc")
        if layout == "oi":
            nc.gpsimd.dma_start(out=xcc[:], in_=src[cc * 128:(cc + 1) * 128, :, :])
        else:
            nc.gpsimd.dma_start(
                out=xcc[:],
                in_=src[:, cc * 128:(cc + 1) * 128, :].rearrange("k i d -> i k d"))
        for n in range(48):
            nc.vector.bn_stats(out=s6[:, cc, n, :], in_=xcc[:, n, :])
        # batched stats post-processing for this cc: columns n
        # bn_stats 6-tuple = (cnt, mean, cnt*var) of even / odd elements.
        sl = slice(cc * 48, (cc + 1) * 48)
        me = s6[:, cc, :, 1]
        mo = s6[:, cc, :, 4]
        cve = s6[:, cc, :, 2]
        cvo = s6[:, cc, :, 5]
        nc.vector.tensor_add(out=tA[:, sl], in0=me, in1=mo)
        nc.vector.tensor_scalar_mul(mean[:, sl], tA[:, sl], 0.5)
        nc.vector.tensor_scalar_mul(negm[:, sl], mean[:, sl], -1.0)
        # var = (cve+cvo)/128 + (me^2+mo^2)/2 - mean^2
        nc.vector.tensor_add(out=tA[:, sl], in0=cve, in1=cvo)
        nc.vector.tensor_scalar_mul(tA[:, sl], tA[:, sl], 1.0 / 128.0)
        nc.vector.tensor_mul(out=tB[:, sl], in0=me, in1=me)
        nc.vector.scalar_tensor_tensor(
            out=tA[:, sl], in0=tB[:, sl], scalar=0.5, in1=tA[:, sl],
            op0=ALU.mult, op1=ALU.add)
        nc.vector.tensor_mul(out=tB[:, sl], in0=mo, in1=mo)
        nc.vector.scalar_tensor_tensor(
            out=tA[:, sl], in0=tB[:, sl], scalar=0.5, in1=tA[:, sl],
            op0=ALU.mult, op1=ALU.add)
        nc.vector.tensor_mul(out=tB[:, sl], in0=mean[:, sl], in1=mean[:, sl])
        nc.vector.tensor_sub(out=tA[:, sl], in0=tA[:, sl], in1=tB[:, sl])
        # rs = 1/sqrt(var+eps)
        nc.vector.tensor_scalar_add(tA[:, sl], tA[:, sl], EPS)
        nc.scalar.sqrt(out=tB[:, sl], in_=tA[:, sl])
        nc.vector.reciprocal(out=rs[:, sl], in_=tB[:, sl])
        # apply (in place), then ONE batched xbar transpose for the chunk:
        # out[p, n, q] = in[q, n*128 + p]  (per-128-block transpose)
        for n in range(48):
            col = cc * 48 + n
            nc.vector.tensor_scalar(
                out=xcc[:, n, :], in0=xcc[:, n, :],
                scalar1=negm[:, col:col + 1], scalar2=rs[:, col:col + 1],
                op0=ALU.add, op1=ALU.mult)
        nc.sync.dma_start(
            out=lnfm.rearrange("p (n j) -> p n j", n=48)[:, :, cc * 128:(cc + 1) * 128],
            in_=xcc.rearrange("p n j -> p (n j)"), transpose=True)


@functools.lru_cache(maxsize=4)
def build_program(has_bo: bool):
    nc = bacc.Bacc(num_devices=N_CORES)

    xr = nc.declare_dram_parameter("xr", [L, NL, D], F32, isOutput=False)
    xc = nc.declare_dram_parameter("xc", [NL, L, D], F32, isOutput=False)
    xb = nc.declare_dram_parameter("xb", [L, NL, D], F32, isOutput=False)
    wq = nc.declare_dram_parameter("wq", [D, D], BF16, isOutput=False)
    wk = nc.declare_dram_parameter("wk", [D, D], BF16, isOutput=False)
    wv = nc.declare_dram_parameter("wv", [D, D], BF16, isOutput=False)
    wg = nc.declare_dram_parameter("wg", [D, D], BF16, isOutput=False)
    wo = nc.declare_dram_parameter("wo", [D, D], BF16, isOutput=False)
    wb = nc.declare_dram_parameter("wb", [D, H], BF16, isOutput=False)
    cq = nc.declare_dram_parameter("cq", [D, 1], F32, isOutput=False)
    ck = nc.declare_dram_parameter("ck", [D, 1], F32, isOutput=False)
    cv = nc.declare_dram_parameter("cv", [D, 1], F32, isOutput=False)
    cg = nc.declare_dram_parameter("cg", [D, 1], F32, isOutput=False)
    cb = nc.declare_dram_parameter("cb", [4, 1], F32, isOutput=False)
    bo_b = nc.declare_dram_parameter("bo_b", [D, D], F32, isOutput=False)
    out = nc.declare_dram_parameter("out", [NL, L, D], F32, isOutput=True)

    a_part = nc.dram_tensor("a_part", [H, L, L], F32)
    a_red = nc.dram_tensor("a_red", [H, L, L], F32, addr_space="Shared")
    bp_part = nc.dram_tensor("bp_part", [H, NL, L], F32)
    bp_gath = nc.dram_tensor("bp_gath", [N_CORES, H, NL, L], F32,
                             addr_space="Shared")

    with TileContext(nc) as tc, ExitStack() as es:
        cpool = es.enter_context(tc.tile_pool(name="consts", bufs=1))
        wq_sb = cpool.tile([D, D], BF16, tag="wq")
        wk_sb = cpool.tile([D, D], BF16, tag="wk")
        wv_sb = cpool.tile([D, D], BF16, tag="wv")
        wg_sb = cpool.tile([D, D], BF16, tag="wg")
        wo_sb = cpool.tile([D, D], BF16, tag="wo")
        wb_sb = cpool.tile([D, H], BF16, tag="wb")
        cq_sb = cpool.tile([D, 1], F32, tag="cq")
        ck_sb = cpool.tile([D, 1], F32, tag="ck")
        cv_sb = cpool.tile([D, 1], F32, tag="cv")
        cg_sb = cpool.tile([D, 1], F32, tag="cg")
        for t, s in [(wq_sb, wq), (wk_sb, wk), (wv_sb, wv), (wg_sb, wg),
                     (wo_sb, wo), (wb_sb, wb), (cq_sb, cq), (ck_sb, ck),
                     (cv_sb, cv), (cg_sb, cg)]:
            nc.sync.dma_start(out=t[:], in_=s[:])
        if has_bo:
            bo_sb = cpool.tile([D, D], F32, tag="bo")
            nc.sync.dma_start(out=bo_sb[:], in_=bo_b[:])

        s6p = es.enter_context(tc.tile_pool(name="s6", bufs=1))
        mvp = es.enter_context(tc.tile_pool(name="mv", bufs=1))
        xccp = es.enter_context(tc.tile_pool(name="xcc", bufs=2))
        lnp = es.enter_context(tc.tile_pool(name="lnfm", bufs=1))
        bigp = es.enter_context(tc.tile_pool(name="big", bufs=1))
        ln_pools = (s6p, mvp, xccp)

        # ---- phase 1: LN of pair rows (n-shard) -> lnfm1
        lnfm1 = lnp.tile([128, NPOS], BF16, tag="lnfm")
        _emit_ln(nc, ln_pools, xr, "oi", lnfm1, "s6")

        # ---- phase 2: q/k/v projections (feature-major)
        q_sb = bigp.tile([128, NPOS], BF16, tag="bigq")
        k_sb = bigp.tile([128, NPOS], BF16, tag="bigk")
        v_pm = bigp.tile([128, NL * NCHUNK, D], BF16, tag="bigvpm")
        with tc.tile_pool(name="ppsum", bufs=2, space="PSUM") as ppsum, \
             tc.tile_pool(name="vrot", bufs=4) as vrotp:
            for dst, w_sb, c_sb, eng in ((q_sb, wq_sb, cq_sb, "act"),
                                         (k_sb, wk_sb, ck_sb, "act")):
                for ch in range(NPOS // 512):
                    ps = ppsum.tile([128, 512], F32, tag="pps")
                    sl = slice(ch * 512, (ch + 1) * 512)
                    nc.tensor.matmul(ps[:], lhsT=w_sb[:], rhs=lnfm1[:, sl],
                                     start=True, stop=True)
                    if eng == "act":
                        nc.scalar.activation(dst[:, sl], ps[:], ACTF.Identity,
                                             bias=c_sb[:, 0:1])
                    else:
                        nc.vector.tensor_scalar_add(dst[:, sl], ps[:],
                                                    c_sb[:, 0:1])
            # v: groups of 2 n-rows; batched xbar transpose per group
            for n2 in range(NL // 2):
                ps2 = ppsum.tile([128, 2, 512], F32, tag="vps")
                for g in range(2):
                    n = n2 * 2 + g
                    nc.tensor.matmul(ps2[:, g, :L],
                                     lhsT=wv_sb[:],
                                     rhs=lnfm1[:, n * L:(n + 1) * L],
                                     start=True, stop=True)
                vr2 = vrotp.tile([128, 2, L], BF16, tag="vrot")
                nc.scalar.activation(vr2[:], ps2[:, :, :L], ACTF.Identity,
                                     bias=cv_sb[:, 0:1])
                nc.sync.dma_start(
                    out=v_pm[:, n2 * 6:(n2 + 1) * 6, :],
                    in_=vr2.rearrange("p n j -> p (n j)"), transpose=True)

        # ---- phase 3: LN bias shard -> lnfm2; bproj position-major
        lnfm2 = lnp.tile([128, NPOS], BF16, tag="lnfm")
        _emit_ln(nc, ln_pools, xb, "oi", lnfm2, "s6")
        bp_sb = bigp.tile([128, NPOS // 128, H], F32, tag="bpsb")
        with tc.tile_pool(name="bpps", bufs=4, space="PSUM") as bpps:
            for tg in range(NPOS // 512):
                ps = bpps.tile([128, 4, H], F32, tag="bpps")
                for c4 in range(4):
                    t = tg * 4 + c4
                    nc.tensor.matmul(ps[:, c4, :],
                                     lhsT=lnfm2[:, t * 128:(t + 1) * 128],
                                     rhs=wb_sb[:], start=True, stop=True)
                nc.vector.tensor_copy(bp_sb[:, tg * 4:(tg + 1) * 4, :], ps[:])
        # stage to DRAM: bp_part[h, il, j]; chunk t=(il, jc), partition=j%128
        for h in range(H):
            nc.sync.dma_start(
                out=bp_part[h].rearrange("il (jc p) -> p il jc", p=128),
                in_=bp_sb[:, :, h].rearrange("p (il jc) -> p il jc", jc=NCHUNK))

        # ---- phase 4: scores A[i, j] per head, K=32 row-tiled over n
        with tc.tile_pool(name="apsum", bufs=2, space="PSUM") as apsum, \
             tc.tile_pool(name="asb", bufs=2) as asbp:
            for ic in range(NCHUNK):
                aps = [apsum.tile([128, L], F32, tag=f"A{h}", name=f"A{h}") for h in range(H)]
                for n in range(NL):
                    for h in range(H):
                        nc.tensor.matmul(
                            aps[h][:],
                            lhsT=q_sb[32 * h:32 * (h + 1),
                                      n * L + ic * 128: n * L + (ic + 1) * 128],
                            rhs=k_sb[32 * h:32 * (h + 1), n * L:(n + 1) * L],
                            start=(n == 0), stop=(n == NL - 1),
                            tile_position=(32 * h, 0))
                a_sb = asbp.tile([128, H, L], F32, tag="asb")
                for h in range(H):
                    if h % 2 == 0:
                        nc.vector.tensor_copy(a_sb[:, h, :], aps[h][:])
                    else:
                        nc.scalar.copy(a_sb[:, h, :], aps[h][:])
                nc.sync.dma_start(
                    out=a_part[:, ic * 128:(ic + 1) * 128, :]
                        .rearrange("h i j -> i h j"),
                    in_=a_sb[:])

        # ---- phase 5: collectives
        nc.gpsimd.collective_compute(
            "AllReduce", ALU.add, replica_groups=RG,
            ins=[a_part[:]], outs=[a_red[:]])
        nc.gpsimd.collective_compute(
            "AllGather", ALU.bypass, replica_groups=RG,
            ins=[bp_part[:]], outs=[bp_gath[:]])

        # ---- phase 6: gate (overlaps collectives via deps)
        lnfm3 = lnp.tile([128, NPOS], BF16, tag="lnfm")
        _emit_ln(nc, ln_pools, xc, "io", lnfm3, "s6")
        gate_sb = bigp.tile([128, NPOS], BF16, tag="bigq")
        with tc.tile_pool(name="gpsum", bufs=4, space="PSUM") as gpsum:
            for ch in range(NPOS // 512):
                ps = gpsum.tile([128, 512], F32, tag="gps")
                sl = slice(ch * 512, (ch + 1) * 512)
                nc.tensor.matmul(ps[:], lhsT=wg_sb[:], rhs=lnfm3[:, sl],
                                 start=True, stop=True)
                nc.scalar.activation(gate_sb[:, sl], ps[:], ACTF.Sigmoid,
                                     bias=cg_sb[:, 0:1])

        # ---- phase 7: softmax (redundant on every core) + attn transpose
        attnT = bigp.tile([128, H * NCHUNK, L], BF16, tag="bigattnT")
        with tc.tile_pool(name="smp", bufs=1) as smp, \
             tc.tile_pool(name="sms", bufs=2) as sms:
            for ic in range(NCHUNK):
                i0 = ic * 128
                a_ch = smp.tile([128, H, L], F32, tag="ach")
                nc.sync.dma_start(
                    out=a_ch[:],
                    in_=a_red[:, i0:i0 + 128, :].rearrange("h i j -> i h j"))
                bp_ch = smp.tile([128, H, L], F32, tag="bpch")
                for cpr in range(N_CORES):
                    lo = max(cpr * NL, i0)
                    hi = min((cpr + 1) * NL, i0 + 128)
                    if lo >= hi:
                        continue
                    nc.sync.dma_start(
                        out=bp_ch[lo - i0:hi - i0, :, :],
                        in_=bp_gath[cpr, :, lo - cpr * NL:hi - cpr * NL, :]
                            .rearrange("h il j -> il h j"))
                # A += bp (+cb per head, baked as floats = 0 in practice)
                nc.vector.tensor_add(out=a_ch[:], in0=a_ch[:], in1=bp_ch[:])
                nm = sms.tile([128, H], F32, tag="nm")
                nc.vector.tensor_reduce(out=nm[:], in_=a_ch[:], axis=AX.X,
                                        op=ALU.max)
                nmn = sms.tile([128, H], F32, tag="nmn")
                nc.vector.tensor_scalar_mul(nmn[:], nm[:], -1.0)
                e_ch = smp.tile([128, H, L], F32, tag="bpch")
                ssum = sms.tile([128, H], F32, tag="ssum")
                for h in range(H):
                    nc.scalar.activation(e_ch[:, h, :], a_ch[:, h, :], ACTF.Exp,
                                         bias=nmn[:, h:h + 1],
                                         accum_out=ssum[:, h:h + 1])
                rsum = sms.tile([128, H], F32, tag="rsum")
                nc.vector.reciprocal(out=rsum[:], in_=ssum[:])
                at_ch = smp.tile([128, H, L], BF16, tag="ach")
                for h in range(H):
                    nc.vector.tensor_scalar_mul(at_ch[:, h, :], e_ch[:, h, :],
                                                rsum[:, h:h + 1])
                nc.sync.dma_start(
                    out=attnT[:, :, i0:i0 + 128],
                    in_=at_ch.rearrange("p h j -> p (h j)"), transpose=True)

        # ---- phase 8: output einsum (col-tiled by head) + gate + out proj
        with tc.tile_pool(name="opsum", bufs=2, space="PSUM") as opsum, \
             tc.tile_pool(name="fpsum", bufs=2, space="PSUM") as fpsum, \
             tc.tile_pool(name="ogp", bufs=3) as ogp, \
             tc.tile_pool(name="fsbp", bufs=2) as fsbp:
            for k in range(NL):
                ops_t = opsum.tile([128, L], F32, tag="ops")
                for jc in range(NCHUNK):
                    for h in range(H):
                        nc.tensor.matmul(
                            ops_t[32 * h:32 * (h + 1), :],
                            lhsT=v_pm[:, k * NCHUNK + jc, 32 * h:32 * (h + 1)],
                            rhs=attnT[:, h * NCHUNK + jc, :],
                            start=(jc == 0), stop=(jc == NCHUNK - 1),
                            tile_position=(0, 32 * h))
                og = ogp.tile([128, L], BF16, tag="og")
                nc.vector.scalar_tensor_tensor(
                    out=og[:], in0=ops_t[:], scalar=1.0,
                    in1=gate_sb[:, k * L:(k + 1) * L],
                    op0=ALU.mult, op1=ALU.mult)
                fps = fpsum.tile([128, NCHUNK, D], F32, tag="fps")
                for pc in range(NCHUNK):
                    nc.tensor.matmul(fps[:, pc, :],
                                     lhsT=og[:, pc * 128:(pc + 1) * 128],
                                     rhs=wo_sb[:], start=True, stop=True)
                if k % 4 == 0:
                    fsb4 = fsbp.tile([128, 4, NCHUNK, D], F32, tag="fsb")
                kk = k % 4
                if has_bo:
                    for pc in range(NCHUNK):
                        nc.vector.tensor_add(out=fsb4[:, kk, pc, :],
                                             in0=fps[:, pc, :], in1=bo_sb[:])
                elif k % 2 == 0:
                    nc.vector.tensor_copy(fsb4[:, kk, :, :], fps[:])
                else:
                    nc.scalar.copy(fsb4[:, kk, :, :], fps[:])
                if kk == 3:
                    k0 = k - 3
                    nc.sync.dma_start(
                        out=out[k0:k0 + 4]
                            .rearrange("k (pc p) d -> p (k pc) d", p=128),
                        in_=fsb4.rearrange("p k pc d -> p (k pc) d"))

    nc.compile()
    return nc


def _prep_inputs(pair, bias, ln_pair_w, ln_pair_b, ln_bias_w, ln_bias_b,
                 Wq, Wk, Wv, Wb, Wg, bg, Wo, bo):
    bf = ml_dtypes.bfloat16
    scaling = 1.0 / math.sqrt(DH)
    kscale = 1.0 / math.sqrt(L)
    wq_e = (ln_pair_w[:, None] * Wq * scaling).astype(bf)
    wk_e = (ln_pair_w[:, None] * Wk * kscale).astype(bf)
    wv_e = (ln_pair_w[:, None] * Wv).astype(bf)
    wg_e = (ln_pair_w[:, None] * Wg).astype(bf)
    wb_e = (ln_bias_w[:, None] * Wb).astype(bf)
    wo_e = Wo.astype(bf)
    cq_e = (ln_pair_b @ (Wq * scaling)).astype(np.float32).reshape(D, 1)
    ck_e = (ln_pair_b @ (Wk * kscale)).astype(np.float32).reshape(D, 1)
    cv_e = (ln_pair_b @ Wv).astype(np.float32).reshape(D, 1)
    cg_e = (bg + ln_pair_b @ Wg).astype(np.float32).reshape(D, 1)
    cb_e = (ln_bias_b @ Wb).astype(np.float32).reshape(H, 1)
    bo_f = np.asarray(bo, np.float32)
    has_bo = bool(np.any(bo_f != 0.0))
    bo_bcast = np.broadcast_to(bo_f, (D, D)).copy() if has_bo \
        else np.zeros((D, D), np.float32)

    common = dict(wq=wq_e, wk=wk_e, wv=wv_e, wg=wg_e, wo=wo_e, wb=wb_e,
                  cq=cq_e, ck=ck_e, cv=cv_e, cg=cg_e, cb=cb_e, bo_b=bo_bcast)
    in_maps = []
    for c in range(N_CORES):
        r0 = c * NL
        m = dict(common)
        m["xr"] = np.ascontiguousarray(pair[0, :, r0:r0 + NL, :], np.float32)
        m["xc"] = np.ascontiguousarray(pair[0, r0:r0 + NL, :, :], np.float32)
        m["xb"] = np.ascontiguousarray(bias[0, :, r0:r0 + NL, :], np.float32)
        in_maps.append(m)
    return in_maps, has_bo


TRACE = False
LAST_EXEC_NS = None
LAST_TRACE_DIR = None


def kernel(**inputs):
    global LAST_EXEC_NS, LAST_TRACE_DIR
    inputs = {k: np.asarray(v) for k, v in inputs.items()}
    in_maps, has_bo = _prep_inputs(**inputs)
    nc = build_program(has_bo)
    res = run_bass_kernel_spmd(nc, in_maps, list(range(N_CORES)), trace=TRACE)
    if TRACE:
        LAST_EXEC_NS = res.exec_time_ns
    full = np.concatenate([res.results[c]["out"] for c in range(N_CORES)],
                          axis=0)[None]
    return full.astype(np.float32)


if __name__ == "__main__":
    nc = build_program(False)
    print("build ok")


# revision 12
# speedup vs baseline: 2.1472x; 1.0636x over previous
"""Biased axial (tied) attention kernel for 8 Trainium2 NeuronCores.

Sharding: the score einsum contracts over the first L axis (n) of the
LN'd/transposed pair tensor.  Each core takes 48 of the 384 n-rows,
computes partial scores A[h,i,j] for ALL (i,j), and the partials are
summed with an on-chip AllReduce (2.4 MB).  The per-(i,j) bias
projection is sharded over i and exchanged with a small AllGather.
After the reduce every core redundantly softmaxes the full score
tensor and computes output columns k in its own n-shard (out[:,k] only
needs attn rows (all i) and locally-projected V rows), so the output
rows of the final (transposed) result are shard-contiguous.

Compute layout: LayerNorm runs position-major (positions on SBUF
partitions) with bn_stats; normalized bf16 tiles are flipped to
feature-major via DMA-xbar transposes; all matmuls run in bf16 with
fp32 PSUM accumulation.  Scores use K=32 row-tiled accumulation (4
heads concurrent in the PE array); the output einsum uses M=32
col-tiled matmuls per head from a j-partition-major V.
"""

import functools
import math
from contextlib import ExitStack

import numpy as np
import ml_dtypes

import concourse.bacc as bacc
import concourse.mybir as mybir
from concourse.bass_utils import run_bass_kernel_spmd
from concourse.tile import TileContext

N_CORES = 8
L = 384
D = 128
H = 4
DH = 32
NL = L // N_CORES          # 48 rows per core
NCHUNK = L // 128          # 3
NPOS = L * NL              # 18432 positions per LN'd tensor slice
EPS = 1e-5

F32 = mybir.dt.float32
BF16 = mybir.dt.bfloat16
AX = mybir.AxisListType
ALU = mybir.AluOpType
ACTF = mybir.ActivationFunctionType

RG = [list(range(N_CORES))]


def _emit_ln(nc, pools, src, layout, lnfm, s6tag):
    """LayerNorm `src` (DRAM f32) position-major, write bf16 feature-major
    into SBUF tile `lnfm` [128, NPOS] (pos = inner*384 + outer).

    layout 'oi': src [L, NL, D] (partition axis 0, group axis 1)
    layout 'io': src [NL, L, D] (partition axis 1, group axis 0)
    """
    s6p, mvp, xccp = pools
    s6 = s6p.tile([128, NCHUNK, 48, 6], F32, tag=s6tag)
    mean = mvp.tile([128, NCHUNK * 48], F32, tag="mean")
    negm = mvp.tile([128, NCHUNK * 48], F32, tag="negm")
    tA = mvp.tile([128, NCHUNK * 48], F32, tag="tA")
    tB = mvp.tile([128, NCHUNK * 48], F32, tag="tB")
    rs = mvp.tile([128, NCHUNK * 48], F32, tag="rs")
    for cc in range(NCHUNK):
        xcc = xccp.tile([128, 48, D], BF16, tag="xcc")
        if layout == "oi":
            nc.gpsimd.dma_start(out=xcc[:], in_=src[cc * 128:(cc + 1) * 128, :, :])
        else:
            nc.gpsimd.dma_start(
                out=xcc[:],
                in_=src[:, cc * 128:(cc + 1) * 128, :].rearrange("k i d -> i k d"))
        for n in range(48):
            nc.vector.bn_stats(out=s6[:, cc, n, :], in_=xcc[:, n, :])
        # batched stats post-processing for this cc: columns n
        # bn_stats 6-tuple = (cnt, mean, cnt*var) of even / odd elements.
        sl = slice(cc * 48, (cc + 1) * 48)
        me = s6[:, cc, :, 1]
        mo = s6[:, cc, :, 4]
        cve = s6[:, cc, :, 2]
        cvo = s6[:, cc, :, 5]
        nc.vector.tensor_add(out=tA[:, sl], in0=me, in1=mo)
        nc.vector.tensor_scalar_mul(mean[:, sl], tA[:, sl], 0.5)
        nc.vector.tensor_scalar_mul(negm[:, sl], mean[:, sl], -1.0)
        # var = (cve+cvo)/128 + (me^2+mo^2)/2 - mean^2
        nc.vector.tensor_add(out=tA[:, sl], in0=cve, in1=cvo)
        nc.vector.tensor_scalar_mul(tA[:, sl], tA[:, sl], 1.0 / 128.0)
        nc.vector.tensor_mul(out=tB[:, sl], in0=me, in1=me)
        nc.vector.scalar_tensor_tensor(
            out=tA[:, sl], in0=tB[:, sl], scalar=0.5, in1=tA[:, sl],
            op0=ALU.mult, op1=ALU.add)
        nc.vector.tensor_mul(out=tB[:, sl], in0=mo, in1=mo)
        nc.vector.scalar_tensor_tensor(
            out=tA[:, sl], in0=tB[:, sl], scalar=0.5, in1=tA[:, sl],
            op0=ALU.mult, op1=ALU.add)
        nc.vector.tensor_mul(out=tB[:, sl], in0=mean[:, sl], in1=mean[:, sl])
        nc.vector.tensor_sub(out=tA[:, sl], in0=tA[:, sl], in1=tB[:, sl])
        # rs = 1/sqrt(var+eps)
        nc.vector.tensor_scalar_add(tA[:, sl], tA[:, sl], EPS)
        nc.scalar.sqrt(out=tB[:, sl], in_=tA[:, sl])
        nc.vector.reciprocal(out=rs[:, sl], in_=tB[:, sl])
        nc.vector.tensor_mul(out=tB[:, sl], in0=negm[:, sl], in1=rs[:, sl])
        # apply (in place), then ONE batched xbar transpose for the chunk:
        # out[p, n, q] = in[q, n*128 + p]  (per-128-block transpose)
        for n in range(48):
            col = cc * 48 + n
            if n % 3 == 2:
                nc.scalar.activation(
                    xcc[:, n, :], xcc[:, n, :], ACTF.Identity,
                    bias=tB[:, col:col + 1], scale=rs[:, col:col + 1])
            else:
                nc.vector.tensor_scalar(
                    out=xcc[:, n, :], in0=xcc[:, n, :],
                    scalar1=negm[:, col:col + 1], scalar2=rs[:, col:col + 1],
                    op0=ALU.add, op1=ALU.mult)
        nc.sync.dma_start(
            out=lnfm.rearrange("p (n j) -> p n j", n=48)[:, :, cc * 128:(cc + 1) * 128],
            in_=xcc.rearrange("p n j -> p (n j)"), transpose=True)


@functools.lru_cache(maxsize=4)
def build_program(has_bo: bool):
    nc = bacc.Bacc(num_devices=N_CORES)

    xr = nc.declare_dram_parameter("xr", [L, NL, D], F32, isOutput=False)
    xc = nc.declare_dram_parameter("xc", [L, NL, D], F32, isOutput=False)
    xb = nc.declare_dram_parameter("xb", [L, NL, D], F32, isOutput=False)
    wq = nc.declare_dram_parameter("wq", [D, D], BF16, isOutput=False)
    wk = nc.declare_dram_parameter("wk", [D, D], BF16, isOutput=False)
    wv = nc.declare_dram_parameter("wv", [D, D], BF16, isOutput=False)
    wg = nc.declare_dram_parameter("wg", [D, D], BF16, isOutput=False)
    wo = nc.declare_dram_parameter("wo", [D, D], BF16, isOutput=False)
    wb = nc.declare_dram_parameter("wb", [D, H], BF16, isOutput=False)
    cq = nc.declare_dram_parameter("cq", [D, 1], F32, isOutput=False)
    ck = nc.declare_dram_parameter("ck", [D, 1], F32, isOutput=False)
    cv = nc.declare_dram_parameter("cv", [D, 1], F32, isOutput=False)
    cg = nc.declare_dram_parameter("cg", [D, 1], F32, isOutput=False)
    cb = nc.declare_dram_parameter("cb", [4, 1], F32, isOutput=False)
    bo_b = nc.declare_dram_parameter("bo_b", [D, D], F32, isOutput=False)
    out = nc.declare_dram_parameter("out", [NL, L, D], F32, isOutput=True)

    a_part = nc.dram_tensor("a_part", [H, L, L], F32)
    a_red = nc.dram_tensor("a_red", [H, L, L], F32, addr_space="Shared")
    bp_part = nc.dram_tensor("bp_part", [H, NL, L], F32)
    bp_gath = nc.dram_tensor("bp_gath", [N_CORES, H, NL, L], F32,
                             addr_space="Shared")

    with TileContext(nc) as tc, ExitStack() as es:
        cpool = es.enter_context(tc.tile_pool(name="consts", bufs=1))
        wq_sb = cpool.tile([D, D], BF16, tag="wq")
        wk_sb = cpool.tile([D, D], BF16, tag="wk")
        wv_sb = cpool.tile([D, D], BF16, tag="wv")
        wg_sb = cpool.tile([D, D], BF16, tag="wg")
        wo_sb = cpool.tile([D, D], BF16, tag="wo")
        wb_sb = cpool.tile([D, H], BF16, tag="wb")
        cq_sb = cpool.tile([D, 1], F32, tag="cq")
        ck_sb = cpool.tile([D, 1], F32, tag="ck")
        cv_sb = cpool.tile([D, 1], F32, tag="cv")
        cg_sb = cpool.tile([D, 1], F32, tag="cg")
        for t, s in [(wq_sb, wq), (wk_sb, wk), (wv_sb, wv), (wg_sb, wg),
                     (wo_sb, wo), (wb_sb, wb), (cq_sb, cq), (ck_sb, ck),
                     (cv_sb, cv), (cg_sb, cg)]:
            nc.sync.dma_start(out=t[:], in_=s[:])
        if has_bo:
            bo_sb = cpool.tile([D, D], F32, tag="bo")
            nc.sync.dma_start(out=bo_sb[:], in_=bo_b[:])

        s6p = es.enter_context(tc.tile_pool(name="s6", bufs=1))
        mvp = es.enter_context(tc.tile_pool(name="mv", bufs=1))
        xccp = es.enter_context(tc.tile_pool(name="xcc", bufs=2))
        lnp = es.enter_context(tc.tile_pool(name="lnfm", bufs=1))
        bigp = es.enter_context(tc.tile_pool(name="big", bufs=1))
        ln_pools = (s6p, mvp, xccp)

        # ---- phase 1: LN of pair rows (n-shard) -> lnfm1
        lnfm1 = lnp.tile([128, NPOS], BF16, tag="lnfm")
        _emit_ln(nc, ln_pools, xr, "oi", lnfm1, "s6")

        # ---- phase 2: q/k/v projections (feature-major)
        q_sb = bigp.tile([128, NPOS], BF16, tag="bigq")
        k_sb = bigp.tile([128, NPOS], BF16, tag="bigk")
        v_pm = bigp.tile([128, NL * NCHUNK, D], BF16, tag="bigvpm")
        with tc.tile_pool(name="ppsum", bufs=3, space="PSUM") as ppsum, \
             tc.tile_pool(name="vrot", bufs=4) as vrotp:
            for dst, w_sb, c_sb, eng in ((q_sb, wq_sb, cq_sb, "act"),
                                         (k_sb, wk_sb, ck_sb, "dve")):
                for ch in range(NPOS // 512):
                    ps = ppsum.tile([128, 512], F32, tag="pps")
                    sl = slice(ch * 512, (ch + 1) * 512)
                    nc.tensor.matmul(ps[:], lhsT=w_sb[:], rhs=lnfm1[:, sl],
                                     start=True, stop=True)
                    if eng == "act":
                        nc.scalar.activation(dst[:, sl], ps[:], ACTF.Identity,
                                             bias=c_sb[:, 0:1])
                    else:
                        nc.vector.tensor_scalar_add(dst[:, sl], ps[:],
                                                    c_sb[:, 0:1])
            # v: groups of 2 n-rows; batched xbar transpose per group
            for n2 in range(NL // 2):
                ps2 = ppsum.tile([128, 2, 512], F32, tag="vps", bufs=2)
                for g in range(2):
                    n = n2 * 2 + g
                    nc.tensor.matmul(ps2[:, g, :L],
                                     lhsT=wv_sb[:],
                                     rhs=lnfm1[:, n * L:(n + 1) * L],
                                     start=True, stop=True)
                vr2 = vrotp.tile([128, 2, L], BF16, tag="vrot")
                nc.scalar.activation(vr2[:], ps2[:, :, :L], ACTF.Identity,
                                     bias=cv_sb[:, 0:1])
                nc.sync.dma_start(
                    out=v_pm[:, n2 * 6:(n2 + 1) * 6, :],
                    in_=vr2.rearrange("p n j -> p (n j)"), transpose=True)

        # ---- phase 3: LN bias shard -> lnfm2; bproj position-major
        lnfm2 = lnp.tile([128, NPOS], BF16, tag="lnfm")
        _emit_ln(nc, ln_pools, xb, "oi", lnfm2, "s6")
        bp_sb = bigp.tile([128, NPOS // 128, H], F32, tag="bpsb")
        with tc.tile_pool(name="bpps", bufs=4, space="PSUM") as bpps:
            for tg in range(NPOS // 512):
                ps = bpps.tile([128, 4, H], F32, tag="bpps")
                for c4 in range(4):
                    t = tg * 4 + c4
                    nc.tensor.matmul(ps[:, c4, :],
                                     lhsT=lnfm2[:, t * 128:(t + 1) * 128],
                                     rhs=wb_sb[:], start=True, stop=True)
                nc.vector.tensor_copy(bp_sb[:, tg * 4:(tg + 1) * 4, :], ps[:])
        # stage to DRAM: bp_part[h, il, j]; chunk t=(il, jc), partition=j%128
        for h in range(H):
            nc.sync.dma_start(
                out=bp_part[h].rearrange("il (jc p) -> p il jc", p=128),
                in_=bp_sb[:, :, h].rearrange("p (il jc) -> p il jc", jc=NCHUNK))

        # ---- phase 4: scores A[i, j] per head, K=32 row-tiled over n
        with tc.tile_pool(name="apsum", bufs=2, space="PSUM") as apsum, \
             tc.tile_pool(name="asb", bufs=2) as asbp:
            for ic in range(NCHUNK):
                aps = [apsum.tile([128, L], F32, tag=f"A{h}", name=f"A{h}") for h in range(H)]
                for n in range(NL):
                    for h in range(H):
                        nc.tensor.matmul(
                            aps[h][:],
                            lhsT=q_sb[32 * h:32 * (h + 1),
                                      n * L + ic * 128: n * L + (ic + 1) * 128],
                            rhs=k_sb[32 * h:32 * (h + 1), n * L:(n + 1) * L],
                            start=(n == 0), stop=(n == NL - 1),
                            tile_position=(32 * h, 0))
                a_sb = asbp.tile([128, H, L], F32, tag="asb")
                for h in range(H):
                    if h % 2 == 0:
                        nc.vector.tensor_copy(a_sb[:, h, :], aps[h][:])
                    else:
                        nc.scalar.copy(a_sb[:, h, :], aps[h][:])
                nc.sync.dma_start(
                    out=a_part[:, ic * 128:(ic + 1) * 128, :]
                        .rearrange("h i j -> i h j"),
                    in_=a_sb[:])

        # ---- phase 6: gate (overlaps collectives via deps)
        lnfm3 = lnp.tile([128, NPOS], BF16, tag="lnfm")
        _emit_ln(nc, ln_pools, xc, "oi", lnfm3, "s6")
        gate_sb = bigp.tile([128, NPOS], BF16, tag="bigq")
        with tc.tile_pool(name="gpsum", bufs=4, space="PSUM") as gpsum:
            for ch in range(NPOS // 512):
                ps = gpsum.tile([128, 512], F32, tag="gps")
                sl = slice(ch * 512, (ch + 1) * 512)
                nc.tensor.matmul(ps[:], lhsT=wg_sb[:], rhs=lnfm3[:, sl],
                                 start=True, stop=True)
                nc.scalar.activation(gate_sb[:, sl], ps[:], ACTF.Sigmoid,
                                     bias=cg_sb[:, 0:1])

        # ---- phase 5: collectives (AllGather first: bp ready early)
        nc.gpsimd.collective_compute(
            "AllGather", ALU.bypass, replica_groups=RG,
            ins=[bp_part[:]], outs=[bp_gath[:]])
        nc.gpsimd.collective_compute(
            "AllReduce", ALU.add, replica_groups=RG,
            ins=[a_part[:]], outs=[a_red[:]])

        # ---- phase 7: softmax (redundant on every core) + attn transpose
        attnT = bigp.tile([128, H * NCHUNK, L], BF16, tag="bigattnT")
        with tc.tile_pool(name="smp", bufs=1) as smp, \
             tc.tile_pool(name="sms", bufs=2) as sms:
            for ic in range(NCHUNK):
                i0 = ic * 128
                a_ch = smp.tile([128, H, L], F32, tag="ach")
                nc.sync.dma_start(
                    out=a_ch[:],
                    in_=a_red[:, i0:i0 + 128, :].rearrange("h i j -> i h j"))
                bp_ch = smp.tile([128, H, L], F32, tag="bpch")
                for cpr in range(N_CORES):
                    lo = max(cpr * NL, i0)
                    hi = min((cpr + 1) * NL, i0 + 128)
                    if lo >= hi:
                        continue
                    nc.sync.dma_start(
                        out=bp_ch[lo - i0:hi - i0, :, :],
                        in_=bp_gath[cpr, :, lo - cpr * NL:hi - cpr * NL, :]
                            .rearrange("h il j -> il h j"))
                # A += bp (+cb per head, baked as floats = 0 in practice)
                nc.vector.tensor_add(out=a_ch[:], in0=a_ch[:], in1=bp_ch[:])
                nm = sms.tile([128, H], F32, tag="nm")
                nc.vector.tensor_reduce(out=nm[:], in_=a_ch[:], axis=AX.X,
                                        op=ALU.max)
                nmn = sms.tile([128, H], F32, tag="nmn")
                nc.vector.tensor_scalar_mul(nmn[:], nm[:], -1.0)
                e_ch = smp.tile([128, H, L], F32, tag="bpch")
                ssum = sms.tile([128, H], F32, tag="ssum")
                for h in range(H):
                    nc.scalar.activation(e_ch[:, h, :], a_ch[:, h, :], ACTF.Exp,
                                         bias=nmn[:, h:h + 1],
                                         accum_out=ssum[:, h:h + 1])
                rsum = sms.tile([128, H], F32, tag="rsum")
                nc.vector.reciprocal(out=rsum[:], in_=ssum[:])
                at_ch = smp.tile([128, H, L], BF16, tag="ach")
                for h in range(H):
                    nc.vector.tensor_scalar_mul(at_ch[:, h, :], e_ch[:, h, :],
                                                rsum[:, h:h + 1])
                nc.sync.dma_start(
                    out=attnT[:, :, i0:i0 + 128],
                    in_=at_ch.rearrange("p h j -> p (h j)"), transpose=True)

        # ---- phase 8: output einsum (col-tiled by head) + gate + out proj
        with tc.tile_pool(name="opsum", bufs=2, space="PSUM") as opsum, \
             tc.tile_pool(name="fpsum", bufs=2, space="PSUM") as fpsum, \
             tc.tile_pool(name="ogp", bufs=3) as ogp, \
             tc.tile_pool(name="fsbp", bufs=2) as fsbp:
            for k in range(NL):
                ops_t = opsum.tile([128, L], F32, tag="ops")
                for jc in range(NCHUNK):
                    for h in range(H):
                        nc.tensor.matmul(
                            ops_t[32 * h:32 * (h + 1), :],
                            lhsT=v_pm[:, k * NCHUNK + jc, 32 * h:32 * (h + 1)],
                            rhs=attnT[:, h * NCHUNK + jc, :],
                            start=(jc == 0), stop=(jc == NCHUNK - 1),
                            tile_position=(0, 32 * h))
                og = ogp.tile([128, L], BF16, tag="og")
                nc.vector.scalar_tensor_tensor(
                    out=og[:], in0=ops_t[:], scalar=1.0,
                    in1=gate_sb[:, k * L:(k + 1) * L],
                    op0=ALU.mult, op1=ALU.mult)
                fps = fpsum.tile([128, NCHUNK, D], F32, tag="fps")
                for pc in range(NCHUNK):
                    nc.tensor.matmul(fps[:, pc, :],
                                     lhsT=og[:, pc * 128:(pc + 1) * 128],
                                     rhs=wo_sb[:], start=True, stop=True)
                if k % 4 == 0:
                    fsb4 = fsbp.tile([128, 4, NCHUNK, D], F32, tag="fsb")
                kk = k % 4
                if has_bo:
                    for pc in range(NCHUNK):
                        nc.vector.tensor_add(out=fsb4[:, kk, pc, :],
                                             in0=fps[:, pc, :], in1=bo_sb[:])
                elif k % 2 == 0:
                    nc.vector.tensor_copy(fsb4[:, kk, :, :], fps[:])
                else:
                    nc.scalar.copy(fsb4[:, kk, :, :], fps[:])
                if kk == 3:
                    k0 = k - 3
                    nc.sync.dma_start(
                        out=out[k0:k0 + 4]
                            .rearrange("k (pc p) d -> p (k pc) d", p=128),
                        in_=fsb4.rearrange("p k pc d -> p (k pc) d"))

    nc.compile()
    return nc


def _prep_inputs(pair, bias, ln_pair_w, ln_pair_b, ln_bias_w, ln_bias_b,
                 Wq, Wk, Wv, Wb, Wg, bg, Wo, bo):
    bf = ml_dtypes.bfloat16
    scaling = 1.0 / math.sqrt(DH)
    kscale = 1.0 / math.sqrt(L)
    wq_e = (ln_pair_w[:, None] * Wq * scaling).astype(bf)
    wk_e = (ln_pair_w[:, None] * Wk * kscale).astype(bf)
    wv_e = (ln_pair_w[:, None] * Wv).astype(bf)
    wg_e = (ln_pair_w[:, None] * Wg).astype(bf)
    wb_e = (ln_bias_w[:, None] * Wb).astype(bf)
    wo_e = Wo.astype(bf)
    cq_e = (ln_pair_b @ (Wq * scaling)).astype(np.float32).reshape(D, 1)
    ck_e = (ln_pair_b @ (Wk * kscale)).astype(np.float32).reshape(D, 1)
    cv_e = (ln_pair_b @ Wv).astype(np.float32).reshape(D, 1)
    cg_e = (bg + ln_pair_b @ Wg).astype(np.float32).reshape(D, 1)
    cb_e = (ln_bias_b @ Wb).astype(np.float32).reshape(H, 1)
    bo_f = np.asarray(bo, np.float32)
    has_bo = bool(np.any(bo_f != 0.0))
    bo_bcast = np.broadcast_to(bo_f, (D, D)).copy() if has_bo \
        else np.zeros((D, D), np.float32)

    common = dict(wq=wq_e, wk=wk_e, wv=wv_e, wg=wg_e, wo=wo_e, wb=wb_e,
                  cq=cq_e, ck=ck_e, cv=cv_e, cg=cg_e, cb=cb_e, bo_b=bo_bcast)
    in_maps = []
    for c in range(N_CORES):
        r0 = c * NL
        m = dict(common)
        m["xr"] = np.ascontiguousarray(pair[0, :, r0:r0 + NL, :], np.float32)
        m["xc"] = np.ascontiguousarray(
            pair[0, r0:r0 + NL, :, :].transpose(1, 0, 2), np.float32)
        m["xb"] = np.ascontiguousarray(bias[0, :, r0:r0 + NL, :], np.float32)
        in_maps.append(m)
    return in_maps, has_bo


TRACE = False
LAST_EXEC_NS = None
LAST_TRACE_DIR = None


def kernel(**inputs):
    global LAST_EXEC_NS, LAST_TRACE_DIR
    inputs = {k: np.asarray(v) for k, v in inputs.items()}
    in_maps, has_bo = _prep_inputs(**inputs)
    nc = build_program(has_bo)
    res = run_bass_kernel_spmd(nc, in_maps, list(range(N_CORES)), trace=TRACE)
    if TRACE:
        LAST_EXEC_NS = res.exec_time_ns
    full = np.concatenate([res.results[c]["out"] for c in range(N_CORES)],
                          axis=0)[None]
    return full.astype(np.float32)


if __name__ == "__main__":
    nc = build_program(False)
    print("build ok")
